# revision 21
# baseline (speedup 1.0000x reference)
"""Trainium2 Bass kernel for nn_Attn_Pred_Model (causal geometric-decay FIR + position biases).

Math:
  out[b,t,d] = alpha * sum_{i=0}^{P-1} beta^i * x[b,t-1-i,d]
               + pos_fwd[d] + pos_bwd[bucket(t,d)]

The FIR along the sequence dim is a banded (block-bidiagonal) Toeplitz matmul:
with 128-row sequence blocks,  y[blk] = D @ x[blk] + L @ x[blk-1]
for two constant 128x128 matrices D, L built from (alpha, beta) on the host.
The (S, 32) position bias is precomputed on the host and added on the
vector engine after the PE matmuls.

Sharding: pure data parallelism — batch dim split across the 8 NeuronCores.
The device-side layout is (S, B_loc, NB): the shard handed to each core is a
transposed *view*; the SPMD runner's input-concat materializes it (same
one-copy cost as contiguous sharding) and in exchange every DMA descriptor
is a 2-16KB contiguous run instead of 128B, which is the difference between
~170 GB/s and ~line-rate HBM bandwidth per core.
"""

import os
import sys

import numpy as np

os.environ.setdefault("MYCRO_LOCAL_CACHE", "1")
if "/opt/trn_rl_repo" not in sys.path:
    sys.path.insert(0, "/opt/trn_rl_repo")

B, S, NB = 1024, 1024, 32
NCORES = 8
B_LOC = B // NCORES  # batches per core
SB = 128             # sequence block size
NTB = S // SB        # sequence blocks
BC = 16              # batches per matmul chunk -> N = BC*NB = 512 columns
NCHUNK_FULL = B_LOC // BC
F32 = np.float32

_PROGRAM_CACHE = {}


def _install_ntff_shim():
    """Provide antenv.axon_hooks if the image lacks it, so trace=True works.

    The axon boot module ships a ctypes NTFF-profile hook but only registers
    it when ``antenv.axon_hooks`` exists; this image's antenv does not have
    that module, which makes ``run_bass_kernel_spmd(trace=True)`` crash on
    import. Inject an in-memory equivalent. No-op if tracing is never used.
    """
    try:
        import antenv.axon_hooks  # noqa: F401
        return
    except ImportError:
        pass
    try:
        import types

        import antenv
        from trn_agent_boot.trn_boot import _ntff_profile_via_ctypes

        hook = _ntff_profile_via_ctypes("/opt/axon/libaxon_pjrt.so")
        mod = types.ModuleType("antenv.axon_hooks")
        state = {"hook": hook}
        mod.get_axon_ntff_profile_hook = lambda: state["hook"]
        mod.set_axon_ntff_profile_hook = lambda h: state.__setitem__("hook", h)
        sys.modules["antenv.axon_hooks"] = mod
        antenv.axon_hooks = mod
    except Exception:
        pass


def _split_multi_waits(nc, maxw=1):
    """Work around a walrus limit in this image: instructions carrying more
    than ~2 sem waits die in codegen with "Too many sync wait commands".
    Move excess waits onto same-engine NoOps placed just before the
    instruction (identical sync semantics, negligible cost)."""
    import concourse.mybir as mybir

    for fn in nc.m.functions:
        for blk in fn.blocks:
            out = []
            changed = False
            for inst in blk.instructions:
                si = inst.sync_info
                if si is not None and len(si.on_wait) > maxw:
                    waits = list(si.on_wait)
                    excess, keep = waits[:-maxw], waits[-maxw:]
                    for k, w in enumerate(excess):
                        out.append(mybir.InstNoOp(
                            name=f"{inst.name}-sw{k}",
                            engine=inst.engine,
                            bass_nofuse=True,
                            sync_info=mybir.SyncInfo(on_wait=[w], on_update=[]),
                        ))
                    inst.sync_info = mybir.SyncInfo(
                        on_wait=list(keep), on_update=list(si.on_update))
                    changed = True
                out.append(inst)
            if changed:
                blk.instructions = out
    return nc


def build_program(b_loc=B_LOC, split_waits=True):
    """Per-core Bass/Tile program. Device-side x/out layout is (S, b_loc, NB).

    split_waits=True post-processes for the HW compiler; pass False when the
    module is destined for CoreSim (the sim rejects the injected NoOps)."""
    import concourse.bass as bass
    import concourse.mybir as mybir
    import concourse.tile as tile

    f32 = mybir.dt.float32
    f32r = mybir.dt.float32r
    nchunk = b_loc // BC

    nc = bass.Bass("TRN2")
    # x and the FIR matrices are float32r (same 4-byte storage, host
    # pre-rounded): fp32r matmul streams 1 col/cycle vs 4 for fp32 LOW_HIGH
    x_h = nc.declare_dram_parameter("x", [S, b_loc, NB], f32r, False)
    dt_h = nc.declare_dram_parameter("dmat", [SB, SB], f32r, False)   # D^T
    lt_h = nc.declare_dram_parameter("lmat", [SB, SB], f32r, False)   # L^T
    pb_h = nc.declare_dram_parameter("pbias", [NTB, SB, NB], f32, False)
    out_h = nc.declare_dram_parameter("out", [S, b_loc, NB], f32, True)

    with tile.TileContext(nc) as tc:
        with (
            tc.tile_pool(name="consts", bufs=1) as cpool,
            tc.tile_pool(name="xin", bufs=4) as xpool,
            tc.tile_pool(name="outp", bufs=3) as opool,
            tc.tile_pool(name="psum", bufs=8, space="PSUM") as ppool,
        ):
            dt_sb = cpool.tile([SB, SB], f32r, tag="dt")
            lt_sb = cpool.tile([SB, SB], f32r, tag="lt")
            pb_sb = cpool.tile([SB, NTB, NB], f32, tag="pb")
            nc.sync.dma_start(dt_sb[:], dt_h[:])
            nc.sync.dma_start(lt_sb[:], lt_h[:])
            nc.sync.dma_start(pb_sb[:], pb_h[:].rearrange("t p d -> p t d"))

            prev_xt = None
            for tb in range(NTB):
                xt = xpool.tile([SB, b_loc, NB], f32r, tag="xt")
                nc.sync.dma_start(xt[:], x_h[tb * SB:(tb + 1) * SB])
                ot = opool.tile([SB, b_loc, NB], f32, tag="ot")
                bias = pb_sb[:, tb:tb + 1, :].broadcast_to((SB, BC, NB))
                # group matmuls by stationary weight (D then L) in halves of
                # 4 PSUM banks: 2-4 LDWEIGHTS per tb instead of one per MM
                for g0 in range(0, nchunk, 4):
                    chunks = range(g0, min(g0 + 4, nchunk))
                    pss = []
                    for c in chunks:
                        bs = slice(c * BC, (c + 1) * BC)
                        ps = ppool.tile([SB, BC, NB], f32, tag="ps")
                        nc.tensor.matmul(ps[:], dt_sb[:], xt[:, bs, :],
                                         start=True, stop=(tb == 0))
                        pss.append(ps)
                    for i, c in enumerate(chunks):
                        bs = slice(c * BC, (c + 1) * BC)
                        if tb > 0:
                            nc.tensor.matmul(pss[i][:], lt_sb[:],
                                             prev_xt[:, bs, :],
                                             start=False, stop=True)
                        nc.vector.tensor_tensor(ot[:, bs, :], pss[i][:], bias,
                                                mybir.AluOpType.add)
                nc.scalar.dma_start(out_h[tb * SB:(tb + 1) * SB], ot[:])
                prev_xt = xt
    return _split_multi_waits(nc) if split_waits else nc


def round_fp32r(a):
    """Round an fp32 array to fp32r precision (what the PE fp32r datapath
    consumes); storage stays 4-byte fp32 bits."""
    from neuron_dtypes import static_cast_fp32_to_fp32r

    return np.ascontiguousarray(
        static_cast_fp32_to_fp32r(np.ascontiguousarray(a, dtype=F32))
        .view(np.float32))


def host_consts(alpha, beta, pos_fwd_param, pos_bwd_param, past_steps):
    """Precompute D^T, L^T (128x128 FIR block matrices) and the position bias."""
    P = int(np.asarray(past_steps).reshape(-1)[0]) if np.ndim(past_steps) else int(past_steps)
    assert P <= SB, f"past_steps {P} > block size {SB} unsupported"
    a = float(np.asarray(alpha).reshape(-1)[0])
    b = float(np.asarray(beta).reshape(-1)[0])
    w = a * np.power(b, np.arange(P, dtype=np.float64))

    idx = np.arange(SB)
    km = idx[:, None] - idx[None, :]          # t - s
    D = np.where((km >= 1) & (km <= P), w[np.clip(km - 1, 0, P - 1)], 0.0)
    kml = km + SB                             # cross-block: t - s + 128
    L = np.where((kml >= 1) & (kml <= P), w[np.clip(kml - 1, 0, P - 1)], 0.0)
    DT = round_fp32r(D.T.astype(F32))
    LT = round_fp32r(L.T.astype(F32))

    t = np.arange(S)[:, None]
    j = np.arange(NB)[None, :]
    bucket = ((t - NB * j) % S) // NB         # (S, NB)
    pf = np.asarray(pos_fwd_param, dtype=np.float64).reshape(NB)
    pbw = np.asarray(pos_bwd_param, dtype=np.float64).reshape(NB)
    pb = pf[None, :] + pbw[bucket]            # (S, NB)
    pbias = np.ascontiguousarray(pb.reshape(NTB, SB, NB), dtype=F32)
    return DT, LT, pbias


def reference_numpy(x, alpha, beta, pos_fwd_param, pos_bwd_param, past_steps):
    """Float64 host reference (for self-tests)."""
    P = int(past_steps)
    a = float(np.asarray(alpha).reshape(-1)[0])
    b = float(np.asarray(beta).reshape(-1)[0])
    w = a * np.power(b, np.arange(P, dtype=np.float64))
    xf = np.asarray(x, dtype=np.float64)
    Bn, Sn, Dn = xf.shape
    y = np.zeros_like(xf)
    for i in range(P):
        y[:, i + 1:, :] += w[i] * xf[:, :Sn - 1 - i, :]
    t = np.arange(Sn)[:, None]
    j = np.arange(Dn)[None, :]
    bucket = ((t - Dn * j) % Sn) // Dn
    pf = np.asarray(pos_fwd_param, dtype=np.float64).reshape(Dn)
    pbw = np.asarray(pos_bwd_param, dtype=np.float64).reshape(Dn)
    return y + pf[None, :] + pbw[bucket]


def kernel(x, alpha, beta, pos_fwd_param, pos_bwd_param, past_steps):
    _install_ntff_shim()
    from concourse.bass_utils import run_bass_kernel_spmd

    x = np.asarray(x)
    assert x.shape == (B, S, NB), x.shape
    x = round_fp32r(x)  # PE fp32r datapath consumes pre-rounded values
    DT, LT, pbias = host_consts(alpha, beta, pos_fwd_param, pos_bwd_param,
                                past_steps)

    if "hw" not in _PROGRAM_CACHE:
        _PROGRAM_CACHE["hw"] = build_program(B_LOC)
    nc = _PROGRAM_CACHE["hw"]

    core_ids = list(range(NCORES))
    in_maps = [
        {
            # transposed view (S, B_LOC, NB); materialized by the runner's
            # input concat — no extra host copy vs contiguous sharding
            "x": x[i * B_LOC:(i + 1) * B_LOC].transpose(1, 0, 2),
            "dmat": DT,
            "lmat": LT,
            "pbias": pbias,
        }
        for i in core_ids
    ]
    res = run_bass_kernel_spmd(nc, in_maps, core_ids)
    out = np.empty((B, S, NB), dtype=F32)
    for i in core_ids:
        out[i * B_LOC:(i + 1) * B_LOC] = res.results[i]["out"].transpose(1, 0, 2)
    if res.exec_time_ns is not None:
        kernel.last_exec_time_ns = res.exec_time_ns
    kernel.last_results = res
    return out


kernel.last_exec_time_ns = None
kernel.last_results = None


# revision 23
# speedup vs baseline: 1.0316x; 1.0316x over previous
"""Trainium2 Bass kernel for nn_Attn_Pred_Model (causal geometric-decay FIR + position biases).

Math:
  out[b,t,d] = alpha * sum_{i=0}^{P-1} beta^i * x[b,t-1-i,d]
               + pos_fwd[d] + pos_bwd[bucket(t,d)]

The FIR along the sequence dim is a banded (block-bidiagonal) Toeplitz matmul:
with 128-row sequence blocks,  y[blk] = D @ x[blk] + L @ x[blk-1]
for two constant 128x128 matrices D, L built from (alpha, beta) on the host.
The (S, 32) position bias is precomputed on the host and added on the
vector engine after the PE matmuls.

Sharding: pure data parallelism — batch dim split across the 8 NeuronCores.
The device-side layout is (S, B_loc, NB): the shard handed to each core is a
transposed *view*; the SPMD runner's input-concat materializes it (same
one-copy cost as contiguous sharding) and in exchange every DMA descriptor
is a 2-16KB contiguous run instead of 128B, which is the difference between
~170 GB/s and ~line-rate HBM bandwidth per core.
"""

import os
import sys

import numpy as np

os.environ.setdefault("MYCRO_LOCAL_CACHE", "1")
if "/opt/trn_rl_repo" not in sys.path:
    sys.path.insert(0, "/opt/trn_rl_repo")

B, S, NB = 1024, 1024, 32
NCORES = 8
B_LOC = B // NCORES  # batches per core
SB = 128             # sequence block size
NTB = S // SB        # sequence blocks
BC = 16              # batches per matmul chunk -> N = BC*NB = 512 columns
NCHUNK_FULL = B_LOC // BC
F32 = np.float32

_PROGRAM_CACHE = {}


def _install_ntff_shim():
    """Provide antenv.axon_hooks if the image lacks it, so trace=True works.

    The axon boot module ships a ctypes NTFF-profile hook but only registers
    it when ``antenv.axon_hooks`` exists; this image's antenv does not have
    that module, which makes ``run_bass_kernel_spmd(trace=True)`` crash on
    import. Inject an in-memory equivalent. No-op if tracing is never used.
    """
    try:
        import antenv.axon_hooks  # noqa: F401
        return
    except ImportError:
        pass
    try:
        import types

        import antenv
        from trn_agent_boot.trn_boot import _ntff_profile_via_ctypes

        hook = _ntff_profile_via_ctypes("/opt/axon/libaxon_pjrt.so")
        mod = types.ModuleType("antenv.axon_hooks")
        state = {"hook": hook}
        mod.get_axon_ntff_profile_hook = lambda: state["hook"]
        mod.set_axon_ntff_profile_hook = lambda h: state.__setitem__("hook", h)
        sys.modules["antenv.axon_hooks"] = mod
        antenv.axon_hooks = mod
    except Exception:
        pass


def _split_multi_waits(nc, maxw=1):
    """Work around a walrus limit in this image: instructions carrying more
    than ~2 sem waits die in codegen with "Too many sync wait commands".
    Move excess waits onto same-engine NoOps placed just before the
    instruction (identical sync semantics, negligible cost)."""
    import concourse.mybir as mybir

    for fn in nc.m.functions:
        for blk in fn.blocks:
            out = []
            changed = False
            for inst in blk.instructions:
                si = inst.sync_info
                if si is not None and len(si.on_wait) > maxw:
                    waits = list(si.on_wait)
                    excess, keep = waits[:-maxw], waits[-maxw:]
                    for k, w in enumerate(excess):
                        out.append(mybir.InstNoOp(
                            name=f"{inst.name}-sw{k}",
                            engine=inst.engine,
                            bass_nofuse=True,
                            sync_info=mybir.SyncInfo(on_wait=[w], on_update=[]),
                        ))
                    inst.sync_info = mybir.SyncInfo(
                        on_wait=list(keep), on_update=list(si.on_update))
                    changed = True
                out.append(inst)
            if changed:
                blk.instructions = out
    return nc


def build_program(b_loc=B_LOC, split_waits=True):
    """Per-core Bass/Tile program. Device-side x/out layout is (S, b_loc, NB).

    split_waits=True post-processes for the HW compiler; pass False when the
    module is destined for CoreSim (the sim rejects the injected NoOps)."""
    import concourse.bass as bass
    import concourse.mybir as mybir
    import concourse.tile as tile

    f32 = mybir.dt.float32
    f32r = mybir.dt.float32r
    nchunk = b_loc // BC

    nc = bass.Bass("TRN2")
    # x and the FIR matrices are float32r (same 4-byte storage, host
    # pre-rounded): fp32r matmul streams 1 col/cycle vs 4 for fp32 LOW_HIGH
    x_h = nc.declare_dram_parameter("x", [S, b_loc, NB], f32r, False)
    dt_h = nc.declare_dram_parameter("dmat", [SB, SB], f32r, False)   # D^T
    lt_h = nc.declare_dram_parameter("lmat", [SB, SB], f32r, False)   # L^T
    pb_h = nc.declare_dram_parameter("pbias", [NTB, SB, NB], f32, False)
    out_h = nc.declare_dram_parameter("out", [S, b_loc, NB], f32, True)

    with tile.TileContext(nc) as tc:
        with (
            tc.tile_pool(name="consts", bufs=1) as cpool,
            tc.tile_pool(name="xin", bufs=4) as xpool,
            tc.tile_pool(name="outp", bufs=3) as opool,
            tc.tile_pool(name="psum", bufs=8, space="PSUM") as ppool,
        ):
            dt_sb = cpool.tile([SB, SB], f32r, tag="dt")
            lt_sb = cpool.tile([SB, SB], f32r, tag="lt")
            pb_sb = cpool.tile([SB, NTB, NB], f32, tag="pb")
            nc.sync.dma_start(dt_sb[:], dt_h[:])
            nc.sync.dma_start(lt_sb[:], lt_h[:])
            nc.sync.dma_start(pb_sb[:], pb_h[:].rearrange("t p d -> p t d"))

            half = max(1, b_loc // 2)
            prev_xt = None
            for tb in range(NTB):
                xt = xpool.tile([SB, b_loc, NB], f32r, tag="xt")
                # two 1MB transfers instead of one 2MB: paired cores sharing
                # an HBM stack interleave more fairly
                nc.sync.dma_start(xt[:, :half, :],
                                  x_h[tb * SB:(tb + 1) * SB, :half, :])
                nc.sync.dma_start(xt[:, half:, :],
                                  x_h[tb * SB:(tb + 1) * SB, half:, :])
                ot = opool.tile([SB, b_loc, NB], f32, tag="ot")
                bias = pb_sb[:, tb:tb + 1, :].broadcast_to((SB, BC, NB))
                # group matmuls by stationary weight (D then L) in halves of
                # 4 PSUM banks: 2-4 LDWEIGHTS per tb instead of one per MM
                for g0 in range(0, nchunk, 4):
                    chunks = range(g0, min(g0 + 4, nchunk))
                    pss = []
                    for c in chunks:
                        bs = slice(c * BC, (c + 1) * BC)
                        ps = ppool.tile([SB, BC, NB], f32, tag="ps")
                        nc.tensor.matmul(ps[:], dt_sb[:], xt[:, bs, :],
                                         start=True, stop=(tb == 0))
                        pss.append(ps)
                    for i, c in enumerate(chunks):
                        bs = slice(c * BC, (c + 1) * BC)
                        if tb > 0:
                            nc.tensor.matmul(pss[i][:], lt_sb[:],
                                             prev_xt[:, bs, :],
                                             start=False, stop=True)
                        nc.vector.tensor_tensor(ot[:, bs, :], pss[i][:], bias,
                                                mybir.AluOpType.add)
                nc.scalar.dma_start(out_h[tb * SB:(tb + 1) * SB, :half, :],
                                    ot[:, :half, :])
                nc.scalar.dma_start(out_h[tb * SB:(tb + 1) * SB, half:, :],
                                    ot[:, half:, :])
                prev_xt = xt
    return _split_multi_waits(nc) if split_waits else nc


def round_fp32r(a):
    """Round an fp32 array to fp32r precision (what the PE fp32r datapath
    consumes); storage stays 4-byte fp32 bits."""
    from neuron_dtypes import static_cast_fp32_to_fp32r

    return np.ascontiguousarray(
        static_cast_fp32_to_fp32r(np.ascontiguousarray(a, dtype=F32))
        .view(np.float32))


def host_consts(alpha, beta, pos_fwd_param, pos_bwd_param, past_steps):
    """Precompute D^T, L^T (128x128 FIR block matrices) and the position bias."""
    P = int(np.asarray(past_steps).reshape(-1)[0]) if np.ndim(past_steps) else int(past_steps)
    assert P <= SB, f"past_steps {P} > block size {SB} unsupported"
    a = float(np.asarray(alpha).reshape(-1)[0])
    b = float(np.asarray(beta).reshape(-1)[0])
    w = a * np.power(b, np.arange(P, dtype=np.float64))

    idx = np.arange(SB)
    km = idx[:, None] - idx[None, :]          # t - s
    D = np.where((km >= 1) & (km <= P), w[np.clip(km - 1, 0, P - 1)], 0.0)
    kml = km + SB                             # cross-block: t - s + 128
    L = np.where((kml >= 1) & (kml <= P), w[np.clip(kml - 1, 0, P - 1)], 0.0)
    DT = round_fp32r(D.T.astype(F32))
    LT = round_fp32r(L.T.astype(F32))

    t = np.arange(S)[:, None]
    j = np.arange(NB)[None, :]
    bucket = ((t - NB * j) % S) // NB         # (S, NB)
    pf = np.asarray(pos_fwd_param, dtype=np.float64).reshape(NB)
    pbw = np.asarray(pos_bwd_param, dtype=np.float64).reshape(NB)
    pb = pf[None, :] + pbw[bucket]            # (S, NB)
    pbias = np.ascontiguousarray(pb.reshape(NTB, SB, NB), dtype=F32)
    return DT, LT, pbias


def reference_numpy(x, alpha, beta, pos_fwd_param, pos_bwd_param, past_steps):
    """Float64 host reference (for self-tests)."""
    P = int(past_steps)
    a = float(np.asarray(alpha).reshape(-1)[0])
    b = float(np.asarray(beta).reshape(-1)[0])
    w = a * np.power(b, np.arange(P, dtype=np.float64))
    xf = np.asarray(x, dtype=np.float64)
    Bn, Sn, Dn = xf.shape
    y = np.zeros_like(xf)
    for i in range(P):
        y[:, i + 1:, :] += w[i] * xf[:, :Sn - 1 - i, :]
    t = np.arange(Sn)[:, None]
    j = np.arange(Dn)[None, :]
    bucket = ((t - Dn * j) % Sn) // Dn
    pf = np.asarray(pos_fwd_param, dtype=np.float64).reshape(Dn)
    pbw = np.asarray(pos_bwd_param, dtype=np.float64).reshape(Dn)
    return y + pf[None, :] + pbw[bucket]


def kernel(x, alpha, beta, pos_fwd_param, pos_bwd_param, past_steps):
    _install_ntff_shim()
    from concourse.bass_utils import run_bass_kernel_spmd

    x = np.asarray(x)
    assert x.shape == (B, S, NB), x.shape
    x = round_fp32r(x)  # PE fp32r datapath consumes pre-rounded values
    DT, LT, pbias = host_consts(alpha, beta, pos_fwd_param, pos_bwd_param,
                                past_steps)

    if "hw" not in _PROGRAM_CACHE:
        _PROGRAM_CACHE["hw"] = build_program(B_LOC)
    nc = _PROGRAM_CACHE["hw"]

    core_ids = list(range(NCORES))
    in_maps = [
        {
            # transposed view (S, B_LOC, NB); materialized by the runner's
            # input concat — no extra host copy vs contiguous sharding
            "x": x[i * B_LOC:(i + 1) * B_LOC].transpose(1, 0, 2),
            "dmat": DT,
            "lmat": LT,
            "pbias": pbias,
        }
        for i in core_ids
    ]
    res = run_bass_kernel_spmd(nc, in_maps, core_ids)
    out = np.empty((B, S, NB), dtype=F32)
    for i in core_ids:
        out[i * B_LOC:(i + 1) * B_LOC] = res.results[i]["out"].transpose(1, 0, 2)
    if res.exec_time_ns is not None:
        kernel.last_exec_time_ns = res.exec_time_ns
    kernel.last_results = res
    return out


kernel.last_exec_time_ns = None
kernel.last_results = None


# revision 24
# speedup vs baseline: 1.1817x; 1.1455x over previous
"""Trainium2 Bass kernel for nn_Attn_Pred_Model (causal geometric-decay FIR + position biases).

Math:
  out[b,t,d] = alpha * sum_{i=0}^{P-1} beta^i * x[b,t-1-i,d]
               + pos_fwd[d] + pos_bwd[bucket(t,d)]

The FIR along the sequence dim is a banded (block-bidiagonal) Toeplitz matmul:
with 128-row sequence blocks,  y[blk] = D @ x[blk] + L @ x[blk-1]
for two constant 128x128 matrices D, L built from (alpha, beta) on the host.
The (S, 32) position bias is precomputed on the host and added on the
vector engine after the PE matmuls.

Sharding: pure data parallelism — batch dim split across the 8 NeuronCores.
The device-side layout is (S, B_loc, NB): the shard handed to each core is a
transposed *view*; the SPMD runner's input-concat materializes it (same
one-copy cost as contiguous sharding) and in exchange every DMA descriptor
is a 2-16KB contiguous run instead of 128B, which is the difference between
~170 GB/s and ~line-rate HBM bandwidth per core.
"""

import os
import sys

import numpy as np

os.environ.setdefault("MYCRO_LOCAL_CACHE", "1")
if "/opt/trn_rl_repo" not in sys.path:
    sys.path.insert(0, "/opt/trn_rl_repo")

B, S, NB = 1024, 1024, 32
NCORES = 8
B_LOC = B // NCORES  # batches per core
SB = 128             # sequence block size
NTB = S // SB        # sequence blocks
BC = 16              # batches per matmul chunk -> N = BC*NB = 512 columns
NCHUNK_FULL = B_LOC // BC
F32 = np.float32

_PROGRAM_CACHE = {}


def _install_ntff_shim():
    """Provide antenv.axon_hooks if the image lacks it, so trace=True works.

    The axon boot module ships a ctypes NTFF-profile hook but only registers
    it when ``antenv.axon_hooks`` exists; this image's antenv does not have
    that module, which makes ``run_bass_kernel_spmd(trace=True)`` crash on
    import. Inject an in-memory equivalent. No-op if tracing is never used.
    """
    try:
        import antenv.axon_hooks  # noqa: F401
        return
    except ImportError:
        pass
    try:
        import types

        import antenv
        from trn_agent_boot.trn_boot import _ntff_profile_via_ctypes

        hook = _ntff_profile_via_ctypes("/opt/axon/libaxon_pjrt.so")
        mod = types.ModuleType("antenv.axon_hooks")
        state = {"hook": hook}
        mod.get_axon_ntff_profile_hook = lambda: state["hook"]
        mod.set_axon_ntff_profile_hook = lambda h: state.__setitem__("hook", h)
        sys.modules["antenv.axon_hooks"] = mod
        antenv.axon_hooks = mod
    except Exception:
        pass


def _split_multi_waits(nc, maxw=1):
    """Work around a walrus limit in this image: instructions carrying more
    than ~2 sem waits die in codegen with "Too many sync wait commands".
    Move excess waits onto same-engine NoOps placed just before the
    instruction (identical sync semantics, negligible cost)."""
    import concourse.mybir as mybir

    for fn in nc.m.functions:
        for blk in fn.blocks:
            out = []
            changed = False
            for inst in blk.instructions:
                si = inst.sync_info
                if si is not None and len(si.on_wait) > maxw:
                    waits = list(si.on_wait)
                    excess, keep = waits[:-maxw], waits[-maxw:]
                    for k, w in enumerate(excess):
                        out.append(mybir.InstNoOp(
                            name=f"{inst.name}-sw{k}",
                            engine=inst.engine,
                            bass_nofuse=True,
                            sync_info=mybir.SyncInfo(on_wait=[w], on_update=[]),
                        ))
                    inst.sync_info = mybir.SyncInfo(
                        on_wait=list(keep), on_update=list(si.on_update))
                    changed = True
                out.append(inst)
            if changed:
                blk.instructions = out
    return nc


def build_program(b_loc=B_LOC, split_waits=True):
    """Per-core Bass/Tile program. Device-side x/out layout is (S, b_loc, NB).

    split_waits=True post-processes for the HW compiler; pass False when the
    module is destined for CoreSim (the sim rejects the injected NoOps)."""
    import concourse.bass as bass
    import concourse.mybir as mybir
    import concourse.tile as tile

    f32 = mybir.dt.float32
    f32r = mybir.dt.float32r
    nchunk = b_loc // BC

    nc = bass.Bass("TRN2")
    # x and the FIR matrices are float32r (same 4-byte storage, host
    # pre-rounded): fp32r matmul streams 1 col/cycle vs 4 for fp32 LOW_HIGH
    x_h = nc.declare_dram_parameter("x", [S, b_loc, NB], f32r, False)
    dt_h = nc.declare_dram_parameter("dmat", [SB, SB], f32r, False)   # D^T
    lt_h = nc.declare_dram_parameter("lmat", [SB, SB], f32r, False)   # L^T
    pb_h = nc.declare_dram_parameter("pbias", [NTB, SB, NB], f32, False)
    out_h = nc.declare_dram_parameter("out", [S, b_loc, NB], f32, True)

    with tile.TileContext(nc) as tc:
        with (
            tc.tile_pool(name="consts", bufs=1) as cpool,
            tc.tile_pool(name="xin", bufs=4) as xpool,
            tc.tile_pool(name="outp", bufs=3) as opool,
            tc.tile_pool(name="psum", bufs=8, space="PSUM") as ppool,
        ):
            dt_sb = cpool.tile([SB, SB], f32r, tag="dt")
            lt_sb = cpool.tile([SB, SB], f32r, tag="lt")
            pb_sb = cpool.tile([SB, NTB, NB], f32, tag="pb")
            nc.sync.dma_start(dt_sb[:], dt_h[:])
            nc.sync.dma_start(lt_sb[:], lt_h[:])
            nc.sync.dma_start(pb_sb[:], pb_h[:].rearrange("t p d -> p t d"))

            half = max(1, b_loc // 2)
            prev_xt = None
            for tb in range(NTB):
                xt = xpool.tile([SB, b_loc, NB], f32r, tag="xt")
                nc.sync.dma_start(xt[:], x_h[tb * SB:(tb + 1) * SB])
                ot = opool.tile([SB, b_loc, NB], f32, tag="ot")
                bias = pb_sb[:, tb:tb + 1, :].broadcast_to((SB, BC, NB))
                # group matmuls by stationary weight (D then L) in halves of
                # 4 PSUM banks: 2-4 LDWEIGHTS per tb instead of one per MM
                for g0 in range(0, nchunk, 4):
                    chunks = range(g0, min(g0 + 4, nchunk))
                    pss = []
                    for c in chunks:
                        bs = slice(c * BC, (c + 1) * BC)
                        ps = ppool.tile([SB, BC, NB], f32, tag="ps")
                        nc.tensor.matmul(ps[:], dt_sb[:], xt[:, bs, :],
                                         start=True, stop=(tb == 0))
                        pss.append(ps)
                    for i, c in enumerate(chunks):
                        bs = slice(c * BC, (c + 1) * BC)
                        if tb > 0:
                            nc.tensor.matmul(pss[i][:], lt_sb[:],
                                             prev_xt[:, bs, :],
                                             start=False, stop=True)
                        nc.vector.tensor_tensor(ot[:, bs, :], pss[i][:], bias,
                                                mybir.AluOpType.add)
                nc.scalar.dma_start(out_h[tb * SB:(tb + 1) * SB], ot[:])
                prev_xt = xt
    return _split_multi_waits(nc) if split_waits else nc


def round_fp32r(a):
    """Round an fp32 array to fp32r precision (what the PE fp32r datapath
    consumes); storage stays 4-byte fp32 bits."""
    from neuron_dtypes import static_cast_fp32_to_fp32r

    return np.ascontiguousarray(
        static_cast_fp32_to_fp32r(np.ascontiguousarray(a, dtype=F32))
        .view(np.float32))


def host_consts(alpha, beta, pos_fwd_param, pos_bwd_param, past_steps):
    """Precompute D^T, L^T (128x128 FIR block matrices) and the position bias."""
    P = int(np.asarray(past_steps).reshape(-1)[0]) if np.ndim(past_steps) else int(past_steps)
    assert P <= SB, f"past_steps {P} > block size {SB} unsupported"
    a = float(np.asarray(alpha).reshape(-1)[0])
    b = float(np.asarray(beta).reshape(-1)[0])
    w = a * np.power(b, np.arange(P, dtype=np.float64))

    idx = np.arange(SB)
    km = idx[:, None] - idx[None, :]          # t - s
    D = np.where((km >= 1) & (km <= P), w[np.clip(km - 1, 0, P - 1)], 0.0)
    kml = km + SB                             # cross-block: t - s + 128
    L = np.where((kml >= 1) & (kml <= P), w[np.clip(kml - 1, 0, P - 1)], 0.0)
    DT = round_fp32r(D.T.astype(F32))
    LT = round_fp32r(L.T.astype(F32))

    t = np.arange(S)[:, None]
    j = np.arange(NB)[None, :]
    bucket = ((t - NB * j) % S) // NB         # (S, NB)
    pf = np.asarray(pos_fwd_param, dtype=np.float64).reshape(NB)
    pbw = np.asarray(pos_bwd_param, dtype=np.float64).reshape(NB)
    pb = pf[None, :] + pbw[bucket]            # (S, NB)
    pbias = np.ascontiguousarray(pb.reshape(NTB, SB, NB), dtype=F32)
    return DT, LT, pbias


def reference_numpy(x, alpha, beta, pos_fwd_param, pos_bwd_param, past_steps):
    """Float64 host reference (for self-tests)."""
    P = int(past_steps)
    a = float(np.asarray(alpha).reshape(-1)[0])
    b = float(np.asarray(beta).reshape(-1)[0])
    w = a * np.power(b, np.arange(P, dtype=np.float64))
    xf = np.asarray(x, dtype=np.float64)
    Bn, Sn, Dn = xf.shape
    y = np.zeros_like(xf)
    for i in range(P):
        y[:, i + 1:, :] += w[i] * xf[:, :Sn - 1 - i, :]
    t = np.arange(Sn)[:, None]
    j = np.arange(Dn)[None, :]
    bucket = ((t - Dn * j) % Sn) // Dn
    pf = np.asarray(pos_fwd_param, dtype=np.float64).reshape(Dn)
    pbw = np.asarray(pos_bwd_param, dtype=np.float64).reshape(Dn)
    return y + pf[None, :] + pbw[bucket]


def kernel(x, alpha, beta, pos_fwd_param, pos_bwd_param, past_steps):
    _install_ntff_shim()
    from concourse.bass_utils import run_bass_kernel_spmd

    x = np.asarray(x)
    assert x.shape == (B, S, NB), x.shape
    x = round_fp32r(x)  # PE fp32r datapath consumes pre-rounded values
    DT, LT, pbias = host_consts(alpha, beta, pos_fwd_param, pos_bwd_param,
                                past_steps)

    if "hw" not in _PROGRAM_CACHE:
        _PROGRAM_CACHE["hw"] = build_program(B_LOC)
    nc = _PROGRAM_CACHE["hw"]

    core_ids = list(range(NCORES))
    in_maps = [
        {
            # transposed view (S, B_LOC, NB); materialized by the runner's
            # input concat — no extra host copy vs contiguous sharding
            "x": x[i * B_LOC:(i + 1) * B_LOC].transpose(1, 0, 2),
            "dmat": DT,
            "lmat": LT,
            "pbias": pbias,
        }
        for i in core_ids
    ]
    res = run_bass_kernel_spmd(nc, in_maps, core_ids)
    out = np.empty((B, S, NB), dtype=F32)
    for i in core_ids:
        out[i * B_LOC:(i + 1) * B_LOC] = res.results[i]["out"].transpose(1, 0, 2)
    if res.exec_time_ns is not None:
        kernel.last_exec_time_ns = res.exec_time_ns
    kernel.last_results = res
    return out


kernel.last_exec_time_ns = None
kernel.last_results = None
